# revision 1
# baseline (speedup 1.0000x reference)
"""Grouped MLP (MoE expert MLP, ragged token groups) on 8 TRN2 NeuronCores.

Strategy: token-parallel across 4 core-pairs; tensor-parallel (split of
the intermediate dim F) within each pair. Tokens are grouped contiguously
by expert; each expert's group is padded to a multiple of 256 tokens
("units"). Units are assigned to the 4 pairs by a greedy scheduler (with
an always-feasible LPT fallback) that produces a single uniform slot
pattern (slot = run of units computed with one expert's weights), so all
8 cores run ONE fully static SPMD program:

  for slot s (static):  DMA this slot's expert weights (expert id is
      runtime data via a dynamic DRAM offset; weight pools are
      double-buffered so slot s+1 prefetches during slot s compute)
    for each 256-token body in the slot (static):
      DMA xT [128, 8ht, 256] -> fc1 (16 f-tiles x 8 h-tiles matmuls)
      -> Gelu -> fc2 (8 h-tiles x 16 f-tiles) -> yT fp16 partial -> DMA

Everything is statically unrolled, so Tile overlaps all DMA with compute;
there are no dynamic-loop all-engine barriers in the steady state (the
only runtime loop is an outer `reps` loop used for timing).

Each core of a pair computes fc1/fc2 over its half of F and writes fp16
partial fc2 sums; the host adds the two halves and scatters to [T, H].
"""

import numpy as np
import ml_dtypes

import concourse.bass as bass
import concourse.mybir as mybir
import concourse.tile as tile
from concourse import bacc
from concourse.bass_utils import run_bass_kernel_spmd

# Problem shape (fixed by the task).
T, H, F, E = 16384, 1024, 4096, 8
NCORES = 8
NPAIRS = 4
UNIT = 256            # token unit = matmul moving width
HT = H // 128         # 8 h-tiles
FT2 = F // 2 // 128   # 16 f-tiles per core (half of F)
FH = F // 2           # 2048
WSPLIT = 4            # weight DMA split (by f-columns / f-tiles)

_BF16 = mybir.dt.bfloat16
_F16 = mybir.dt.float16
_F32 = mybir.dt.float32
_I32 = mybir.dt.int32

GELU_FUNC = mybir.ActivationFunctionType.Gelu

_cache = {}


def _schedule(counts):
    """counts[E] -> (C, pattern, cells, units_of) for 4 uniform pairs.

    pattern: tuple of slot widths (in units), sum == C
    cells[s][g]: expert id for slot s on pair g
    units_of[g]: list of (expert, tok_start, valid) per chunk j (len C);
                 dummies are (e, -1, 0).
    """
    starts = np.concatenate([[0], np.cumsum(counts)])
    queues = {}
    n = 0
    for e in range(E):
        c = int(counts[e])
        q = []
        for off in range(0, c, UNIT):
            q.append((e, int(starts[e]) + off, min(UNIT, c - off)))
        if q:
            queues[e] = q
            n += len(q)
    if n == 0:
        queues[0] = [(0, -1, 0)]
        n = 1
    C = -(-n // NPAIRS)

    # Greedy slot pattern: each slot assigns (possibly repeated) experts
    # to the 4 pairs; width = what all assigned cells can fill.
    r = {e: len(q) for e, q in queues.items()}
    pattern, cells = [], []
    cap = C
    while cap > 0 and sum(r.values()) > 0:
        order = sorted([e for e in r if r[e] > 0], key=lambda e: -r[e])
        cell = [order[g % len(order)] for g in range(NPAIRS)]
        shares = {}
        for e in cell:
            shares[e] = shares.get(e, 0) + 1
        p = min(max(1, r[e] // shares[e]) for e in set(cell))
        p = max(1, min(p, cap))
        for e in cell:
            r[e] = max(0, r[e] - p)
        pattern.append(p)
        cells.append(cell)
        cap -= p
    if cap > 0 and sum(r.values()) == 0:
        pattern.append(cap)
        cells.append(list(cells[-1]) if cells else [0] * NPAIRS)
    try:
        if sum(r.values()) > 0:
            raise AssertionError("greedy under-served")
        units_of = _deal(queues, pattern, cells)
    except AssertionError:
        pattern, cells = _lpt_pattern(
            {e: len(q) for e, q in queues.items()}, C)
        units_of = _deal(queues, pattern, cells)
    return sum(pattern), tuple(pattern), cells, units_of


def _lpt_pattern(unit_counts, C):
    """Always-feasible: LPT list-schedule runs into 4 groups of capacity C
    (splitting oversized experts), then unify slot boundaries across groups
    (a run split by a boundary just reloads the same expert)."""
    loads = [0] * NPAIRS
    runs = [[] for _ in range(NPAIRS)]
    for e, u in sorted(unit_counts.items(), key=lambda kv: -kv[1]):
        rem = u
        while rem > 0:
            g = min(range(NPAIRS), key=lambda i: loads[i])
            take = min(rem, C - loads[g])
            assert take > 0, "LPT overflow"
            runs[g].append([e, take])
            loads[g] += take
            rem -= take
    any_e = next(iter(unit_counts))
    for g in range(NPAIRS):
        runs[g].sort(key=lambda r_: -r_[1])
        if loads[g] < C:
            if runs[g]:
                runs[g][-1][1] += C - loads[g]
            else:
                runs[g].append([any_e, C])
    bounds = sorted({a for g in range(NPAIRS)
                     for a in np.cumsum([ln for _, ln in runs[g]]).tolist()})
    assert bounds[-1] == C
    pattern = [b - a for a, b in zip([0] + bounds, bounds)]
    cells = []
    pos = 0
    for p in pattern:
        cell = []
        for g in range(NPAIRS):
            acc = 0
            chosen = runs[g][-1][0]
            for e, ln in runs[g]:
                if pos < acc + ln:
                    chosen = e
                    break
                acc += ln
            cell.append(chosen)
        cells.append(cell)
        pos += p
    return pattern, cells


def _deal(queues, pattern, cells):
    """Assign each expert's units to its (pair, slot) cells; pad with
    dummies."""
    pos = {e: 0 for e in queues}
    units_of = [[] for _ in range(NPAIRS)]
    for s, p in enumerate(pattern):
        for g in range(NPAIRS):
            e = cells[s][g]
            for _ in range(p):
                q = queues.get(e, [])
                if pos.get(e, 0) < len(q):
                    units_of[g].append(q[pos[e]])
                    pos[e] += 1
                else:
                    units_of[g].append((e, -1, 0))
    for e, q in queues.items():
        assert pos[e] == len(q), f"unplaced units for expert {e}"
    return units_of


def _build(pattern):
    if pattern in _cache:
        return _cache[pattern]
    C = sum(pattern)
    S = len(pattern)

    nc = bacc.Bacc("TRN2", target_bir_lowering=False, debug=False,
                   num_devices=NCORES)
    xt_d = nc.declare_dram_parameter("xt", [H, C * UNIT], _BF16,
                                     isOutput=False)
    w1_d = nc.declare_dram_parameter("w1", [H, E * FH], _BF16,
                                     isOutput=False)
    w2_d = nc.declare_dram_parameter("w2", [FH, E * H], _BF16,
                                     isOutput=False)
    meta_d = nc.declare_dram_parameter("meta", [1, S + 1], _I32,
                                       isOutput=False)
    yt_d = nc.declare_dram_parameter("yt", [H, C * UNIT], _F16,
                                     isOutput=True)

    xt_r = xt_d.rearrange("(ht p) m -> p ht m", p=128)
    w1_r = w1_d.rearrange("(ht p) m -> p ht m", p=128)
    w2_r = w2_d.rearrange("(ft p) m -> p ft m", p=128)
    yt_r = yt_d.rearrange("(ht p) m -> p ht m", p=128)

    with tile.TileContext(nc) as tc:
        with (
            tc.tile_pool(name="meta", bufs=1) as mpool,
            tc.tile_pool(name="w1", bufs=2) as w1pool,
            tc.tile_pool(name="w2", bufs=2) as w2pool,
            tc.tile_pool(name="x", bufs=2) as xpool,
            tc.tile_pool(name="act", bufs=2) as apool,
            tc.tile_pool(name="y", bufs=2) as ypool,
            tc.tile_pool(name="ps1", bufs=6, space="PSUM") as ps1pool,
            tc.tile_pool(name="ps2", bufs=2, space="PSUM") as ps2pool,
        ):
            mt = mpool.tile([1, S + 1], _I32)
            nc.sync.dma_start(mt[:], meta_d[:])
            reps = nc.values_load(mt[:1, S:S + 1], min_val=1, max_val=100000,
                                  skip_runtime_bounds_check=True)
            w1offs, w2offs = [], []
            for s in range(S):
                # skip_runtime_bounds_check: runtime assert traps kill the
                # axon/PJRT execution path.
                e_s = nc.values_load(mt[:1, s:s + 1], min_val=0,
                                     max_val=E - 1,
                                     skip_runtime_bounds_check=True)
                w1offs.append(nc.s_assert_within(
                    e_s * FH, min_val=0, max_val=(E - 1) * FH,
                    skip_runtime_assert=True))
                w2offs.append(nc.s_assert_within(
                    e_s * H, min_val=0, max_val=(E - 1) * H,
                    skip_runtime_assert=True))

            rep_loop = tc.For_i(0, reps, name="reps")
            rep_loop.__enter__()
            j = 0
            for s in range(S):
                w1sb = w1pool.tile([128, HT, FH], _BF16, tag="w1sb")
                w2sb = w2pool.tile([128, FT2, H], _BF16, tag="w2sb")
                # Split weight loads by f-columns/f-tiles so early fc1/fc2
                # groups can start before the whole slot load lands.
                fq = FH // WSPLIT
                for q in range(WSPLIT):
                    nc.sync.dma_start(
                        w1sb[:, :, q * fq:(q + 1) * fq],
                        w1_r[:, :, bass.ds(w1offs[s] + q * fq, fq)])
                ftq = FT2 // WSPLIT
                for q in range(WSPLIT):
                    nc.sync.dma_start(
                        w2sb[:, q * ftq:(q + 1) * ftq],
                        w2_r[:, q * ftq:(q + 1) * ftq,
                             bass.ds(w2offs[s], H)])
                for _u in range(pattern[s]):
                    col = j * UNIT
                    W = UNIT
                    xt_sb = xpool.tile([128, HT, W], _BF16, tag="xt")
                    nc.sync.dma_start(xt_sb[:], xt_r[:, :, col:col + W])
                    act_sb = apool.tile([128, FT2, W], _BF16, tag="act")
                    for f in range(FT2):
                        ps = ps1pool.tile([128, W], _F32, tag="ps1")
                        for h in range(HT):
                            nc.tensor.matmul(
                                ps[:],
                                w1sb[:, h, f * 128:(f + 1) * 128],
                                xt_sb[:, h],
                                start=(h == 0), stop=(h == HT - 1))
                        nc.scalar.activation(act_sb[:, f], ps[:], GELU_FUNC)
                    yt_sb = ypool.tile([128, HT, W], _F16, tag="yt")
                    for h in range(HT):
                        ps2 = ps2pool.tile([128, W], _F32, tag="ps2")
                        for f in range(FT2):
                            nc.tensor.matmul(
                                ps2[:],
                                w2sb[:, f, h * 128:(h + 1) * 128],
                                act_sb[:, f],
                                start=(f == 0), stop=(f == FT2 - 1))
                        nc.vector.tensor_copy(yt_sb[:, h], ps2[:])
                    nc.sync.dma_start(yt_r[:, :, col:col + W], yt_sb[:])
                    j += 1
            assert j == C
            rep_loop.__exit__(None, None, None)
    nc.compile()
    _cache[pattern] = nc
    return nc


def _make_inputs(x, w1, w2, schedule, reps=1):
    C, pattern, cells, units_of = schedule
    S = len(pattern)
    w1b = w1.astype(ml_dtypes.bfloat16)
    w2b = w2.astype(ml_dtypes.bfloat16)
    # w1 half: [E, H, FH] -> [H, E*FH]; w2 half: [E, FH, H] -> [FH, E*H]
    w1h_ = [np.ascontiguousarray(
        w1b[:, :, half * FH:(half + 1) * FH].transpose(1, 0, 2)
        .reshape(H, E * FH)) for half in range(2)]
    w2h_ = [np.ascontiguousarray(
        w2b[:, half * FH:(half + 1) * FH, :].transpose(1, 0, 2)
        .reshape(FH, E * H)) for half in range(2)]
    in_maps = []
    for pair in range(NPAIRS):
        xt = np.zeros((H, C * UNIT), ml_dtypes.bfloat16)
        for jj, (e, g, v) in enumerate(units_of[pair]):
            if v > 0:
                xt[:, jj * UNIT:jj * UNIT + v] = x[g:g + v].T
        meta = np.zeros((1, S + 1), np.int32)
        for s in range(S):
            meta[0, s] = cells[s][pair]
        meta[0, S] = reps
        for half in range(2):
            in_maps.append({"xt": xt, "w1": w1h_[half], "w2": w2h_[half],
                            "meta": meta})
    return in_maps


def _gather(results, schedule):
    C, pattern, cells, units_of = schedule
    out = np.zeros((T, H), np.float32)
    for pair in range(NPAIRS):
        ya = np.asarray(results[2 * pair]["yt"], np.float32)
        yb = np.asarray(results[2 * pair + 1]["yt"], np.float32)
        ys = ya + yb
        for jj, (e, g, v) in enumerate(units_of[pair]):
            if v > 0:
                out[g:g + v] = ys[:, jj * UNIT:jj * UNIT + v].T
    return out


def prepare(x, w1, w2, counts):
    """For test harness: compiled program + in_maps factory with a reps knob."""
    schedule = _schedule(counts)
    nc = _build(schedule[1])

    def make_in_maps(reps):
        return _make_inputs(x, w1, w2, schedule, reps=reps)

    return nc, make_in_maps


def kernel(permuted_local_hidden_states, weight1, weight2, tokens_per_expert):
    x = np.asarray(permuted_local_hidden_states, np.float32)
    w1 = np.asarray(weight1, np.float32)
    w2 = np.asarray(weight2, np.float32)
    counts = np.asarray(tokens_per_expert).astype(np.int64)

    schedule = _schedule(counts)
    nc = _build(schedule[1])
    in_maps = _make_inputs(x, w1, w2, schedule)
    res = run_bass_kernel_spmd(nc, in_maps, list(range(NCORES)))
    return _gather(res.results, schedule)



# revision 2
# speedup vs baseline: 1.0903x; 1.0903x over previous
"""Grouped MLP (MoE expert MLP, ragged token groups) on 8 TRN2 NeuronCores.

Strategy: 8-way tensor-parallel split of the intermediate dim F. Every
core processes ALL tokens with its F/8 = 512 column slice of w1 (and the
matching 512-row slice of w2), producing a partial fc2 sum; the host adds
the 8 fp16 partials and transposes to [T, H].

Why this layout: tokens are grouped contiguously by expert, so each core
walks experts 0..7 in order over the token stream — expert identity,
weight offsets, and chunk widths are all STATIC (no runtime indexing, no
token scheduling / padding). Chunk widths are exact (<= 512, the PSUM
bank limit), so the PE does exactly T*64 cycles of matmul work per core
-- the bf16 roofline for this decomposition. Per-expert weights are tiny
(2 MB/core), so weight prefetch hides trivially under compute.

  for e in experts (static):   DMA w1/w2 slices (double-buffered)
    for each chunk of expert e's tokens (static, width w <= 512):
      DMA xT [128, 8ht, w] -> fc1 (4 f-tiles x 8 h-accum matmuls)
      -> Gelu -> fc2 (8 h-tiles x 4 f-accum) -> yT fp16 partial -> DMA

Everything is statically unrolled inside a runtime `reps` loop (timing
only); Tile overlaps all DMA with compute.
"""

import numpy as np
import ml_dtypes

import concourse.bass as bass  # noqa: F401  (kept for parity with tooling)
import concourse.mybir as mybir
import concourse.tile as tile
from concourse import bacc
from concourse.bass_utils import run_bass_kernel_spmd

# Problem shape (fixed by the task).
T, H, F, E = 16384, 1024, 4096, 8
NCORES = 8
FS = F // NCORES      # 512: per-core F slice
HT = H // 128         # 8 h-tiles
FT = FS // 128        # 4 f-tiles per core
WMAX = 512            # max matmul moving width (PSUM bank = 512 f32)

_BF16 = mybir.dt.bfloat16
_F16 = mybir.dt.float16
_F32 = mybir.dt.float32
_I32 = mybir.dt.int32

GELU_FUNC = mybir.ActivationFunctionType.Gelu

_cache = {}


def _chunks(counts):
    """counts[E] -> list of (expert, col_start, width) with width <= WMAX.

    Each expert's contiguous token run is split into near-equal chunks, so
    there is no padding at all: sum of widths == sum(counts)."""
    chunks = []
    col = 0
    for e in range(E):
        c = int(counts[e])
        if c <= 0:
            continue
        k = -(-c // WMAX)
        base, rem = divmod(c, k)
        off = 0
        for i in range(k):
            w = base + (1 if i < rem else 0)
            chunks.append((e, col + off, w))
            off += w
        col += c
    return chunks


def _build(counts_key):
    if counts_key in _cache:
        return _cache[counts_key]
    chunks = _chunks(counts_key)

    nc = bacc.Bacc("TRN2", target_bir_lowering=False, debug=False,
                   num_devices=NCORES)
    xt_d = nc.declare_dram_parameter("xt", [H, T], _BF16, isOutput=False)
    w1_d = nc.declare_dram_parameter("w1", [H, E * FS], _BF16,
                                     isOutput=False)
    w2_d = nc.declare_dram_parameter("w2", [FS, E * H], _BF16,
                                     isOutput=False)
    meta_d = nc.declare_dram_parameter("meta", [1, 1], _I32, isOutput=False)
    yt_d = nc.declare_dram_parameter("yt", [H, T], _F16, isOutput=True)

    xt_r = xt_d.rearrange("(ht p) m -> p ht m", p=128)
    w1_r = w1_d.rearrange("(ht p) m -> p ht m", p=128)
    w2_r = w2_d.rearrange("(ft p) m -> p ft m", p=128)
    yt_r = yt_d.rearrange("(ht p) m -> p ht m", p=128)

    with tile.TileContext(nc) as tc:
        with (
            tc.tile_pool(name="meta", bufs=1) as mpool,
            tc.tile_pool(name="w1", bufs=3) as w1pool,
            tc.tile_pool(name="w2", bufs=3) as w2pool,
            tc.tile_pool(name="x", bufs=3) as xpool,
            tc.tile_pool(name="act", bufs=2) as apool,
            tc.tile_pool(name="y", bufs=3) as ypool,
            tc.tile_pool(name="ps1", bufs=4, space="PSUM") as ps1pool,
            tc.tile_pool(name="ps2", bufs=4, space="PSUM") as ps2pool,
        ):
            mt = mpool.tile([1, 1], _I32)
            nc.sync.dma_start(mt[:], meta_d[:])
            # skip_runtime_bounds_check: runtime assert traps kill the
            # axon/PJRT execution path.
            reps = nc.values_load(mt[:1, 0:1], min_val=1, max_val=100000,
                                  skip_runtime_bounds_check=True)

            rep_loop = tc.For_i(0, reps, name="reps")
            rep_loop.__enter__()
            cur_e = None
            w1sb = w2sb = None
            for (e, col, w) in chunks:
                if e != cur_e:
                    cur_e = e
                    w1sb = w1pool.tile([128, HT, FS], _BF16, tag="w1sb")
                    w2sb = w2pool.tile([128, FT, H], _BF16, tag="w2sb")
                    # Split weight loads (parallel DMA queues; lets early
                    # f-tiles start before the whole load lands).
                    fq = FS // 4
                    for q in range(4):
                        nc.sync.dma_start(
                            w1sb[:, :, q * fq:(q + 1) * fq],
                            w1_r[:, :, e * FS + q * fq:e * FS + (q + 1) * fq])
                    for q in range(4):
                        nc.sync.dma_start(
                            w2sb[:, q:q + 1],
                            w2_r[:, q:q + 1, e * H:(e + 1) * H])
                xt_sb = xpool.tile([128, HT, WMAX], _BF16, tag="xt")
                nc.sync.dma_start(xt_sb[:, :, :w], xt_r[:, :, col:col + w])
                act_sb = apool.tile([128, FT, WMAX], _BF16, tag="act")
                for f in range(FT):
                    ps = ps1pool.tile([128, WMAX], _F32, tag="ps1")
                    for h in range(HT):
                        nc.tensor.matmul(
                            ps[:, :w],
                            w1sb[:, h, f * 128:(f + 1) * 128],
                            xt_sb[:, h, :w],
                            start=(h == 0), stop=(h == HT - 1))
                    nc.scalar.activation(act_sb[:, f, :w], ps[:, :w],
                                         GELU_FUNC)
                yt_sb = ypool.tile([128, HT, WMAX], _F16, tag="yt")
                for h in range(HT):
                    ps2 = ps2pool.tile([128, WMAX], _F32, tag="ps2")
                    for f in range(FT):
                        nc.tensor.matmul(
                            ps2[:, :w],
                            w2sb[:, f, h * 128:(h + 1) * 128],
                            act_sb[:, f, :w],
                            start=(f == 0), stop=(f == FT - 1))
                    nc.vector.tensor_copy(yt_sb[:, h, :w], ps2[:, :w])
                nc.sync.dma_start(yt_r[:, :, col:col + w], yt_sb[:, :, :w])
            rep_loop.__exit__(None, None, None)
    nc.compile()
    _cache[counts_key] = nc
    return nc


def _make_inputs(x, w1, w2, reps=1):
    xt = np.ascontiguousarray(x.T.astype(ml_dtypes.bfloat16))
    w1b = w1.astype(ml_dtypes.bfloat16)
    w2b = w2.astype(ml_dtypes.bfloat16)
    meta = np.full((1, 1), reps, np.int32)
    in_maps = []
    for c in range(NCORES):
        # w1 slice: [E, H, FS] -> [H, E*FS]; w2 slice: [E, FS, H] -> [FS, E*H]
        w1c = np.ascontiguousarray(
            w1b[:, :, c * FS:(c + 1) * FS].transpose(1, 0, 2)
            .reshape(H, E * FS))
        w2c = np.ascontiguousarray(
            w2b[:, c * FS:(c + 1) * FS, :].transpose(1, 0, 2)
            .reshape(FS, E * H))
        in_maps.append({"xt": xt, "w1": w1c, "w2": w2c, "meta": meta})
    return in_maps


def _gather(results):
    ys = np.zeros((H, T), np.float32)
    for r in results:
        ys += np.asarray(r["yt"], np.float32)
    return np.ascontiguousarray(ys.T)


def prepare(x, w1, w2, counts):
    """For test harness: compiled program + in_maps factory with a reps knob."""
    key = tuple(int(c) for c in counts)
    nc = _build(key)

    def make_in_maps(reps):
        return _make_inputs(x, w1, w2, reps=reps)

    return nc, make_in_maps


def kernel(permuted_local_hidden_states, weight1, weight2, tokens_per_expert):
    x = np.asarray(permuted_local_hidden_states, np.float32)
    w1 = np.asarray(weight1, np.float32)
    w2 = np.asarray(weight2, np.float32)
    counts = np.asarray(tokens_per_expert).astype(np.int64)
    assert int(counts.sum()) == T, counts

    nc = _build(tuple(int(c) for c in counts))
    in_maps = _make_inputs(x, w1, w2)
    res = run_bass_kernel_spmd(nc, in_maps, list(range(NCORES)))
    return _gather(res.results)


# revision 3
# speedup vs baseline: 1.5895x; 1.4578x over previous
"""Grouped MLP (MoE expert MLP, ragged token groups) on 8 TRN2 NeuronCores.

Strategy: 8-way tensor-parallel split of the intermediate dim F. Every
core processes ALL tokens with its F/8 = 512 column slice of w1 (and the
matching 512-row slice of w2), producing a partial fc2 sum; the host adds
the 8 fp16 partials and transposes to [T, H].

Why this layout: tokens are grouped contiguously by expert, so each core
walks experts 0..7 in order over the token stream — expert identity,
weight offsets, and chunk widths are all STATIC (no runtime indexing, no
token scheduling / padding). Chunk widths are exact (<= 512, the PSUM
bank limit), so the PE does exactly T*64 cycles of matmul work per core
-- the bf16 roofline for this decomposition. Per-expert weights are tiny
(2 MB/core), so weight prefetch hides trivially under compute.

All DRAM tensors are packed chunk-major, [128, ...] with each chunk's
(or expert's) per-partition data one contiguous run (~7-8 KB): DMA
descriptors are fat, which is what the DMA engines need to run at full
rate (short per-row descriptors measured ~3x slower).

  for e in experts (static):   DMA w1/w2 slices (double-buffered)
    for each chunk of expert e's tokens (static, width w <= 512):
      DMA xT [128, 8ht*w] -> fc1 (4 f-tiles x 8 h-accum matmuls)
      -> Gelu -> fc2 (8 h-tiles x 4 f-accum) -> yT fp16 partial -> DMA

Everything is statically unrolled inside a runtime `reps` loop (timing
only); Tile overlaps all DMA with compute.
"""

import numpy as np
import ml_dtypes

import concourse.bass as bass  # noqa: F401  (kept for parity with tooling)
import concourse.mybir as mybir
import concourse.tile as tile
from concourse import bacc
from concourse.bass_utils import run_bass_kernel_spmd

# Problem shape (fixed by the task).
T, H, F, E = 16384, 1024, 4096, 8
NCORES = 8
FS = F // NCORES      # 512: per-core F slice
HT = H // 128         # 8 h-tiles
FT = FS // 128        # 4 f-tiles per core
WMAX = 512            # max matmul moving width (PSUM bank = 512 f32)

_BF16 = mybir.dt.bfloat16
_F16 = mybir.dt.float16
_F32 = mybir.dt.float32
_I32 = mybir.dt.int32

GELU_FUNC = mybir.ActivationFunctionType.Gelu

_cache = {}


def _chunks(counts):
    """counts[E] -> list of (expert, col_start, width) with width <= WMAX.

    Each expert's contiguous token run is split into near-equal chunks, so
    there is no padding at all: sum of widths == sum(counts)."""
    chunks = []
    col = 0
    for e in range(E):
        c = int(counts[e])
        if c <= 0:
            continue
        k = -(-c // WMAX)
        base, rem = divmod(c, k)
        off = 0
        for i in range(k):
            w = base + (1 if i < rem else 0)
            chunks.append((e, col + off, w))
            off += w
        col += c
    return chunks


def _build(counts_key):
    if counts_key in _cache:
        return _cache[counts_key]
    chunks = _chunks(counts_key)

    nc = bacc.Bacc("TRN2", target_bir_lowering=False, debug=False,
                   num_devices=NCORES)
    xt_d = nc.declare_dram_parameter("xt", [128, HT * T], _BF16,
                                     isOutput=False)
    w1_d = nc.declare_dram_parameter("w1", [128, E * HT * FS], _BF16,
                                     isOutput=False)
    w2_d = nc.declare_dram_parameter("w2", [128, E * FT * H], _BF16,
                                     isOutput=False)
    meta_d = nc.declare_dram_parameter("meta", [1, 1], _I32, isOutput=False)
    yt_d = nc.declare_dram_parameter("yt", [128, HT * T], _F16,
                                     isOutput=True)

    with tile.TileContext(nc) as tc:
        with (
            tc.tile_pool(name="meta", bufs=1) as mpool,
            tc.tile_pool(name="w1", bufs=3) as w1pool,
            tc.tile_pool(name="w2", bufs=3) as w2pool,
            tc.tile_pool(name="x", bufs=4) as xpool,
            tc.tile_pool(name="act", bufs=2) as apool,
            tc.tile_pool(name="y", bufs=4) as ypool,
            tc.tile_pool(name="ps1", bufs=4, space="PSUM") as ps1pool,
            tc.tile_pool(name="ps2", bufs=4, space="PSUM") as ps2pool,
        ):
            mt = mpool.tile([1, 1], _I32)
            nc.sync.dma_start(mt[:], meta_d[:])
            # skip_runtime_bounds_check: runtime assert traps kill the
            # axon/PJRT execution path.
            reps = nc.values_load(mt[:1, 0:1], min_val=1, max_val=100000,
                                  skip_runtime_bounds_check=True)

            rep_loop = tc.For_i(0, reps, name="reps")
            rep_loop.__enter__()
            cur_e = None
            w1sb = w2sb = None
            for (e, col, w) in chunks:
                if e != cur_e:
                    cur_e = e
                    w1sb = w1pool.tile([128, HT * FS], _BF16, tag="w1sb")
                    w2sb = w2pool.tile([128, FT * H], _BF16, tag="w2sb")
                    # Split weight loads (parallel DMA + lets the first
                    # h/f tiles start before the whole load lands).
                    wq = HT * FS // 4
                    for q in range(4):
                        nc.sync.dma_start(
                            w1sb[:, q * wq:(q + 1) * wq],
                            w1_d[:, e * HT * FS + q * wq:
                                 e * HT * FS + (q + 1) * wq])
                    for q in range(4):
                        nc.sync.dma_start(
                            w2sb[:, q * wq:(q + 1) * wq],
                            w2_d[:, e * FT * H + q * wq:
                                 e * FT * H + (q + 1) * wq])
                o = HT * col
                xt_sb = xpool.tile([128, HT * w], _BF16, tag="xt")
                nc.sync.dma_start(xt_sb[:, :4 * w], xt_d[:, o:o + 4 * w])
                nc.sync.dma_start(xt_sb[:, 4 * w:], xt_d[:, o + 4 * w:
                                                         o + 8 * w])
                act_sb = apool.tile([128, FT * w], _BF16, tag="act")
                for f in range(FT):
                    ps = ps1pool.tile([128, WMAX], _F32, tag="ps1")
                    for h in range(HT):
                        nc.tensor.matmul(
                            ps[:, :w],
                            w1sb[:, h * FS + f * 128:h * FS + (f + 1) * 128],
                            xt_sb[:, h * w:(h + 1) * w],
                            start=(h == 0), stop=(h == HT - 1))
                    nc.scalar.activation(act_sb[:, f * w:(f + 1) * w],
                                         ps[:, :w], GELU_FUNC)
                yt_sb = ypool.tile([128, HT * w], _F16, tag="yt")
                for h in range(HT):
                    ps2 = ps2pool.tile([128, WMAX], _F32, tag="ps2")
                    for f in range(FT):
                        nc.tensor.matmul(
                            ps2[:, :w],
                            w2sb[:, f * H + h * 128:f * H + (h + 1) * 128],
                            act_sb[:, f * w:(f + 1) * w],
                            start=(f == 0), stop=(f == FT - 1))
                    nc.vector.tensor_copy(yt_sb[:, h * w:(h + 1) * w],
                                          ps2[:, :w])
                nc.sync.dma_start(yt_d[:, o:o + 4 * w], yt_sb[:, :4 * w])
                nc.sync.dma_start(yt_d[:, o + 4 * w:o + 8 * w],
                                  yt_sb[:, 4 * w:])
            rep_loop.__exit__(None, None, None)
    nc.compile()
    _cache[counts_key] = nc
    return nc


def _pack_rows(mat, nt):
    """[nt*128, cols] -> [128, nt*cols] with per-partition [nt, cols]
    contiguous blocks."""
    cols = mat.shape[1]
    return np.ascontiguousarray(
        mat.reshape(nt, 128, cols).transpose(1, 0, 2).reshape(128, nt * cols))


def _make_inputs(x, w1, w2, reps=1):
    xb = x.astype(ml_dtypes.bfloat16)
    w1b = w1.astype(ml_dtypes.bfloat16)
    w2b = w2.astype(ml_dtypes.bfloat16)
    # x: [T, H] -> packed [128, HT*T], token-major per chunk == global
    # token-major: block for token t is [HT] x col t -> pack whole thing
    # as [128, ht, t] contiguous in (ht, t)? No: chunk-major == contiguous
    # token ranges, and within a range [ht][tok] blocks. Global layout
    # [128, ht, T] would interleave ht with FULL T; we need per-chunk
    # blocks, i.e. [128, sum_j (ht * w_j)]. Since chunks tile the token
    # axis in order, pack per chunk below in _pack_x.
    meta = np.full((1, 1), reps, np.int32)
    in_maps = []
    for c in range(NCORES):
        w1c = np.concatenate(
            [_pack_rows(w1b[e, :, c * FS:(c + 1) * FS], HT)
             for e in range(E)], axis=1)
        w2c = np.concatenate(
            [_pack_rows(w2b[e, c * FS:(c + 1) * FS, :], FT)
             for e in range(E)], axis=1)
        in_maps.append({"w1": w1c, "w2": w2c, "meta": meta})
    return xb, in_maps


def _pack_x(xb, chunks):
    xtT = np.ascontiguousarray(xb.T)  # [H, T]
    parts = []
    for (e, col, w) in chunks:
        parts.append(_pack_rows(xtT[:, col:col + w], HT))
    return np.concatenate(parts, axis=1)  # [128, HT*T]


def _gather(results, chunks):
    ys = np.zeros((128, HT * T), np.float32)
    for r in results:
        ys += np.asarray(r["yt"], np.float32)
    out = np.empty((T, H), np.float32)
    for (e, col, w) in chunks:
        o = HT * col
        blk = ys[:, o:o + HT * w].reshape(128, HT, w)
        out[col:col + w] = blk.transpose(2, 1, 0).reshape(w, H)
    return out


def prepare(x, w1, w2, counts):
    """For test harness: compiled program + in_maps factory with a reps knob."""
    key = tuple(int(c) for c in counts)
    nc = _build(key)
    chunks = _chunks(key)

    def make_in_maps(reps):
        xb, in_maps = _make_inputs(x, w1, w2, reps=reps)
        xt = _pack_x(xb, chunks)
        for m in in_maps:
            m["xt"] = xt
        return in_maps

    return nc, make_in_maps


def kernel(permuted_local_hidden_states, weight1, weight2, tokens_per_expert):
    x = np.asarray(permuted_local_hidden_states, np.float32)
    w1 = np.asarray(weight1, np.float32)
    w2 = np.asarray(weight2, np.float32)
    counts = np.asarray(tokens_per_expert).astype(np.int64)
    assert int(counts.sum()) == T, counts

    key = tuple(int(c) for c in counts)
    nc = _build(key)
    chunks = _chunks(key)
    xb, in_maps = _make_inputs(x, w1, w2)
    xt = _pack_x(xb, chunks)
    for m in in_maps:
        m["xt"] = xt
    res = run_bass_kernel_spmd(nc, in_maps, list(range(NCORES)))
    return _gather(res.results, chunks)
